# revision 2
# baseline (speedup 1.0000x reference)
"""MultiHeadAttention Trainium2 kernel: 8-core (batch, head)-sharded.

Sharding: core c handles batch c//4, heads [4*(c%4) .. 4*(c%4)+4).
Each core computes attention for its 4 heads plus its partial (row-parallel)
contribution to the output projection; host sums 4 partials per batch and
adds the bias.

Math (per batch b, head h):
  S = (Q Wq^T + bq)(K Wk^T + bk)^T / 32
    = Q A' K^T + 1 w^T + (terms constant over the softmax axis, dropped)
  with A' = Wq^T Wk / 32,  w = K (Wk^T bq) / 32   (bk cancels in softmax)
  P = softmax(S)  (no max subtraction: |S| <~ 2 for N(0,1)-scale inputs)
  O = P (V Wv^T + bv) = (P V) Wv^T + 1 bv^T
  out = sum_h O_h Wo_h^T + bo  ->  bv folds into bo on host.

v3 design: the kernel is ScalarE-bound -- 128 exp ACTIVATEs (FD=1024,
~1.2us each) are the wall; everything else hides under that stream.
  - Per lk-tile t the emission is [stE_i0, stE_i1, stO_i0, stO_i1],
    [EXP_E(t), EXP_O(t)], fillers.  The E,E prefix means EXP_E(t)'s input
    is ready the moment EXP_O(t-1) retires, so ScalarE never idles; the
    stO pair co-issues on PE row group (64,0) underneath EXP_E(t).
  - Fillers (everything that is not S^T or exp) run strictly AFTER the
    EXPs of their slot so no long-wait op ever blocks the in-order PE
    queue ahead of the critical path: U matmuls of the even head (this
    segment, lagged), U matmuls of the odd head (PREVIOUS segment, spread
    over this segment's scalar-bound phase), tail ops (1/r broadcast via
    K=2 matmul, Wv fold, normalize-STT), and output-projection pieces of
    finished lq-blocks.
  - All matmul/E dtypes bf16 (fp8 E or V injects ~3% noise into the
    signed U sums -- it does NOT average out; measured 3.7e-2 rel err).
  - Segment-boundary work (even-U drain, u eviction) is deferred into the
    next segment's filler slots; the single u PSUM accumulator ping-pongs
    even->odd->even via WAR on its DVE eviction.
"""

import sys

sys.path.insert(0, "/opt/trn_rl_repo")

import numpy as np

HEADS = 16
D_MODEL = 1024
HD = 64
B = 2
L = 2048
NCORES = 8
HPC = 4          # heads per core
PAIRS = 2        # head pairs per core
NLQB = 4         # lq blocks per core (qat projection granularity)
LQB = L // NLQB  # 512
NLKT = L // 128  # 16 lk tiles
BL = 1024        # lq block per segment

_cache = {}


def _build(has_wbias: bool):
    import concourse.bass as bass  # noqa: F401
    import concourse.tile as tile
    from concourse import bacc, mybir

    f32 = mybir.dt.float32
    f32r = mybir.dt.float32r
    bf16 = mybir.dt.bfloat16
    Exp = mybir.ActivationFunctionType.Exp
    mult = mybir.AluOpType.mult

    nc = bacc.Bacc("TRN2", target_bir_lowering=False, debug=False,
                   num_devices=NCORES)

    qt_d = nc.dram_tensor("qt", [128, PAIRS, L], bf16, kind="ExternalInput")
    kt_d = nc.dram_tensor("kt", [128, PAIRS, L], bf16, kind="ExternalInput")
    v_d = nc.dram_tensor("v", [128, HPC, NLKT, 65], bf16,
                         kind="ExternalInput")
    a_d = nc.dram_tensor("a", [128, 128], bf16, kind="ExternalInput")
    wvts_d = nc.dram_tensor("wvts", [64, 2, 128], f32r, kind="ExternalInput")
    ones64_d = nc.dram_tensor("ones64", [65, 2, 128], f32r,
                              kind="ExternalInput")
    wot_d = nc.dram_tensor("wot", [128, PAIRS, D_MODEL], bf16,
                           kind="ExternalInput")
    if has_wbias:
        wb_d = nc.dram_tensor("wb", [128, HPC, NLKT], f32,
                              kind="ExternalInput")
    out_d = nc.dram_tensor("out", [L, D_MODEL], f32, kind="ExternalOutput")

    NSEG = (L // BL) * PAIRS  # 4 segments: (b, p) = (0,0),(0,1),(1,0),(1,1)

    with tile.TileContext(nc) as tc:
        with (
            tc.tile_pool(name="big", bufs=1) as big,
            tc.tile_pool(name="epool", bufs=40) as epool,
            tc.tile_pool(name="small", bufs=2) as small,
            tc.tile_pool(name="stg", bufs=4) as stgp,
            tc.tile_pool(name="stp", bufs=1, space="PSUM") as stp,
            tc.tile_pool(name="up", bufs=1, space="PSUM") as up,
            tc.tile_pool(name="auxp", bufs=2, space="PSUM") as auxp,
        ):
            # ---- loads, earliest-needed first; split so the first
            # segment's compute starts after ~1MB instead of ~3MB
            a_sb = big.tile([128, 128], bf16)
            nc.sync.dma_start(a_sb[:], a_d[:])
            qt_sb = big.tile([128, PAIRS, L], bf16)
            nc.sync.dma_start(qt_sb[:, 0, 0:BL], qt_d[:, 0, 0:BL])
            kt_sb = big.tile([128, PAIRS, L], bf16)
            nc.sync.dma_start(kt_sb[:, 0, :], kt_d[:, 0, :])
            v_sb = big.tile([128, HPC, NLKT, 65], bf16)
            nc.sync.dma_start(v_sb[:, 0:1], v_d[:, 0:1])
            nc.sync.dma_start(qt_sb[:, 0, BL:L], qt_d[:, 0, BL:L])
            nc.sync.dma_start(qt_sb[:, 1, :], qt_d[:, 1, :])
            nc.sync.dma_start(v_sb[:, 1:2], v_d[:, 1:2])
            nc.sync.dma_start(kt_sb[:, 1, :], kt_d[:, 1, :])
            nc.sync.dma_start(v_sb[:, 2:4], v_d[:, 2:4])
            ones64_sb = big.tile([65, 2, 128], f32r)
            nc.sync.dma_start(ones64_sb[:], ones64_d[:])
            wvts_sb = big.tile([64, 2, 128], f32r)
            nc.sync.dma_start(wvts_sb[:], wvts_d[:])
            wot_sb = big.tile([128, PAIRS, D_MODEL], bf16)
            nc.sync.dma_start(wot_sb[:], wot_d[:])
            if has_wbias:
                wb_sb = big.tile([128, HPC, NLKT], f32)
                nc.sync.dma_start(wb_sb[:], wb_d[:])

            # ---- Qa^T = A'-projection of Q^T (block-diagonal A' projects
            # both heads of a pair in one K=128 matmul).  Only the two
            # blocks the first segment needs run upfront; the rest defer
            # into segment 0's filler slots so they never sit ahead of the
            # first S^T matmuls in the PE queue.
            qat_sb = big.tile([128, PAIRS, L], bf16)

            def emit_qat(p, j):
                sl = slice(j * LQB, (j + 1) * LQB)
                qp = auxp.tile([128, LQB], f32, tag="aux",
                               name=f"qp{p}_{j}")
                nc.tensor.matmul(qp[:], a_sb[:], qt_sb[:, p, sl],
                                 start=True, stop=True)
                nc.vector.tensor_copy(qat_sb[:, p, sl], qp[:])

            emit_qat(0, 0)
            emit_qat(0, 1)
            # deferred: (0,2) (0,3) (1,0) (1,1) (1,2) (1,3) in seg-0 slots

            otn_sb = [big.tile([128, L], bf16, tag=f"otn{p}",
                               name=f"otn{p}") for p in range(PAIRS)]

            # ---- tail helpers -------------------------------------------
            def emit_tail_half(pv, bv, un, i):
                """1/r broadcast + Wv fold + normalize, one lq half.

                Denominators live in un[h] row 64 (the V' ones column); two
                K=1 matmuls broadcast r_even into PSUM rows 0:64 and r_odd
                into rows 64:128 (accumulating into one bank), then one DVE
                reciprocal evicts 1/r for the normalize STT.  No partition
                DMA, no [2,BL] staging."""
                csl = slice(i * LQB, (i + 1) * LQB)
                rb = auxp.tile([128, LQB], f32, tag="aux",
                               name=f"rb{bv}_{pv}_{i}")
                nc.tensor.matmul(rb[:], ones64_sb[64:65, 0, :],
                                 un[0][64:65, csl],
                                 start=True, stop=False,
                                 tile_position=(64, 0))
                nc.tensor.matmul(rb[:], ones64_sb[64:65, 1, :],
                                 un[1][64:65, csl],
                                 start=False, stop=True,
                                 tile_position=(64, 0))
                rbs = small.tile([128, LQB], f32, tag="rbs",
                                 name=f"rbs{bv}_{pv}_{i}")
                nc.vector.reciprocal_approx_fast(out=rbs[:], in_=rb[:])
                ot = auxp.tile([128, LQB], f32, tag="aux",
                               name=f"ot{bv}_{pv}_{i}")
                nc.tensor.matmul(ot[:], wvts_sb[:, 0, :],
                                 un[0][0:64, csl], start=True, stop=False)
                nc.tensor.matmul(ot[:], wvts_sb[:, 1, :],
                                 un[1][0:64, csl], start=False, stop=True)
                nc.vector.scalar_tensor_tensor(
                    out=otn_sb[pv][:, bv * BL + i * LQB:
                                   bv * BL + (i + 1) * LQB],
                    in0=ot[:], scalar=1.0, in1=rbs[:], op0=mult, op1=mult)

            def emit_proj_piece(bv, lt, scalar_evict=False):
                """Output projection for one 128-row lq tile of block bv."""
                l0 = bv * BL + lt * 128
                for nh in range(2):
                    nsl = slice(nh * 512, (nh + 1) * 512)
                    pp = auxp.tile([128, 512], f32, tag="aux",
                                   name=f"pp{bv}_{lt}_{nh}")
                    nc.tensor.matmul(pp[:], otn_sb[0][:, l0:l0 + 128],
                                     wot_sb[:, 0, nsl],
                                     start=True, stop=False)
                    nc.tensor.matmul(pp[:], otn_sb[1][:, l0:l0 + 128],
                                     wot_sb[:, 1, nsl],
                                     start=False, stop=True)
                    stg = stgp.tile([128, 512], f32, tag="stg",
                                    name=f"stg{bv}_{lt}_{nh}")
                    if scalar_evict and nh == 0:
                        # post-loop only: ScalarE is idle, let it carry
                        # half the PSUM evictions
                        nc.scalar.copy(stg[:], pp[:])
                    else:
                        nc.vector.tensor_copy(stg[:], pp[:])
                    nc.sync.dma_start(out_d[l0:l0 + 128, nsl], stg[:])

            def emit_u_mms(u, hv, chunks, e_chunks, is_first, is_last):
                """U accumulation matmuls for lk chunks (2 MMs per chunk)."""
                for tc_ in chunks:
                    for i in range(2):
                        isl = slice(i * LQB, (i + 1) * LQB)
                        nc.tensor.matmul(
                            u[:, isl], v_sb[:, hv, tc_, :],
                            e_chunks[tc_][:, isl],
                            start=(is_first and tc_ == 0),
                            stop=(is_last and tc_ == NLKT - 1))

            def finalize_u(u, bv, pv, hh):
                """Evict U rows 0:65 (row 64 = the softmax denominators)."""
                un = small.tile([65, BL], f32r, tag="un",
                                name=f"un{bv}_{pv}_{hh}", bufs=4)
                nc.vector.tensor_copy(un[:], u[0:65, :])
                return un

            # ---- main loop ----------------------------------------------
            # prev: state of segment s-1 {b, p, eE, eO, u_even, drain, un,
            #        u_odd}
            prev = None
            for si in range(NSEG):
                b, p = si // PAIRS, si % PAIRS
                eE = [epool.tile([128, BL], bf16, tag="e",
                                 name=f"eE{b}_{p}_{t}") for t in range(NLKT)]
                eO = [epool.tile([128, BL], bf16, tag="e",
                                 name=f"eO{b}_{p}_{t}") for t in range(NLKT)]
                # even-U chunk schedule: segment 0 has no odd-prev fillers,
                # so it spreads its even U over all slots (denser first
                # half keeps the PE's HAM clock warm); later segments pack
                # it into t=8..14 after the odd-prev U finishes
                if si == 0:
                    even_sched = {t: [t - 1] for t in range(1, NLKT)}
                    drain = [15]
                else:
                    even_sched = {8: [0, 1]}
                    for t_ in range(9, 15):
                        even_sched[t_] = [2 * (t_ - 8), 2 * (t_ - 8) + 1]
                    drain = [14, 15]
                u_even = None
                for t in range(NLKT):
                    # -- S^T row-packed pair matmuls: E,E then O,O
                    stE = stp.tile([128, BL], f32, tag="stE",
                                   name=f"stE{b}_{p}_{t}")
                    stO = stp.tile([128, BL], f32, tag="stO",
                                   name=f"stO{b}_{p}_{t}")
                    ksl = slice(t * 128, (t + 1) * 128)
                    for i in range(2):
                        csl = slice(i * LQB, (i + 1) * LQB)
                        qsl = slice(b * BL + i * LQB, b * BL + (i + 1) * LQB)
                        nc.tensor.matmul(stE[:, csl], kt_sb[0:64, p, ksl],
                                         qat_sb[0:64, p, qsl],
                                         start=True, stop=True,
                                         tile_position=(0, 0))
                    for i in range(2):
                        csl = slice(i * LQB, (i + 1) * LQB)
                        qsl = slice(b * BL + i * LQB, b * BL + (i + 1) * LQB)
                        nc.tensor.matmul(stO[:, csl], kt_sb[64:128, p, ksl],
                                         qat_sb[64:128, p, qsl],
                                         start=True, stop=True,
                                         tile_position=(64, 0))
                    # -- exp fused into the PSUM eviction (ScalarE wall)
                    biasE = (wb_sb[:, 2 * p, t:t + 1] if has_wbias else 0.0)
                    biasO = (wb_sb[:, 2 * p + 1, t:t + 1]
                             if has_wbias else 0.0)
                    nc.scalar.activation(eE[t][:], stE[:], Exp, bias=biasE)
                    nc.scalar.activation(eO[t][:], stO[:], Exp, bias=biasO)
                    # -- fillers (strictly after the EXPs: nothing here may
                    # ever sit ahead of the next S^T in the PE queue)
                    if t == 0:
                        if prev is not None:
                            # drain prev even-U tail chunks + evict
                            emit_u_mms(prev["u_even"], 2 * prev["p"],
                                       prev["drain"], prev["eE"],
                                       False, True)
                            prev["un"] = {0: finalize_u(
                                prev["u_even"], prev["b"], prev["p"], 0)}
                    else:
                        if prev is not None and 1 <= t <= 7:
                            if t == 1:
                                prev["u_odd"] = up.tile(
                                    [65, BL], f32, tag="u",
                                    name=f"uO{prev['b']}_{prev['p']}")
                            emit_u_mms(prev["u_odd"], 2 * prev["p"] + 1,
                                       [2 * (t - 1), 2 * t - 1],
                                       prev["eO"], True, False)
                        if prev is not None and t == 8:
                            emit_u_mms(prev["u_odd"], 2 * prev["p"] + 1,
                                       [14, 15], prev["eO"], False, True)
                            prev["un"][1] = finalize_u(
                                prev["u_odd"], prev["b"], prev["p"], 1)
                        if t in even_sched:
                            if u_even is None:
                                u_even = up.tile([65, BL], f32, tag="u",
                                                 name=f"uE{b}_{p}")
                            emit_u_mms(u_even, 2 * p, even_sched[t], eE,
                                       even_sched[t][0] == 0, False)
                        if si == 0 and 1 <= t <= 6:
                            emit_qat(*[(0, 2), (0, 3), (1, 0), (1, 1),
                                       (1, 2), (1, 3)][t - 1])
                        if prev is not None:
                            if t == 11:
                                emit_tail_half(prev["p"], prev["b"],
                                               prev["un"], 0)
                            elif t == 12:
                                emit_tail_half(prev["p"], prev["b"],
                                               prev["un"], 1)
                            elif t >= 13 and p == 0 and b >= 1:
                                # proj of finished block b-1: lt 0..5 here,
                                # lt 6,7 spill into the next segment
                                for lt in range(2 * (t - 13),
                                                2 * (t - 13) + 2):
                                    emit_proj_piece(b - 1, lt)
                            if p == 1 and b >= 1 and t in (5, 6):
                                emit_proj_piece(b - 1, 1 + t)
                        if si == NSEG - 1 and t == 15:
                            # last segment: drain + evict even U now so
                            # the post-loop odd burst starts immediately
                            emit_u_mms(u_even, 2 * p, drain, eE,
                                       False, True)
                            un_last = {0: finalize_u(u_even, b, p, 0)}
                prev = {"b": b, "p": p, "eE": eE, "eO": eO,
                        "u_even": u_even, "drain": drain, "un": None,
                        "u_odd": None}

            # ---- post-loop: last segment's odd head + tails + proj(1).
            # The odd-U burst runs i=0 then i=1; each un half evicts
            # behind the other half's matmuls so the tail chain never
            # stalls the PE queue.
            prev["un"] = un_last
            u_odd = up.tile([65, BL], f32, tag="u", name="uO_last")
            unO = None
            for i in range(2):
                isl = slice(i * LQB, (i + 1) * LQB)
                for tc_ in range(NLKT):
                    nc.tensor.matmul(u_odd[:, isl],
                                     v_sb[:, 2 * prev["p"] + 1, tc_, :],
                                     prev["eO"][tc_][:, isl],
                                     start=(tc_ == 0), stop=(tc_ == 15))
                if unO is None:
                    unO = small.tile([65, BL], f32r, tag="un",
                                     name="unO_last", bufs=4)
                nc.vector.tensor_copy(unO[:, isl], u_odd[0:65, isl])
            prev["un"][1] = unO
            emit_tail_half(prev["p"], prev["b"], prev["un"], 0)
            for lt in range(0, 4):
                emit_proj_piece(1, lt, scalar_evict=True)
            emit_tail_half(prev["p"], prev["b"], prev["un"], 1)
            for lt in range(4, 8):
                emit_proj_piece(1, lt, scalar_evict=True)
    nc.compile()
    return nc


def _get_nc(has_wbias: bool):
    key = ("nc", has_wbias)
    if key not in _cache:
        _cache[key] = _build(has_wbias)
    return _cache[key]


def _prep_inputs(values, keys, query, Wq, bq, Wk, bk, Wv, bv, Wo, bo):
    """Host-side shard/layout prep. Returns (in_maps, bo_eff, has_wbias)."""
    f32 = np.float32
    values = np.asarray(values, f32)
    keys = np.asarray(keys, f32)
    query = np.asarray(query, f32)
    Wq = np.asarray(Wq, f32)
    bq = np.asarray(bq, f32)
    Wk = np.asarray(Wk, f32)
    bk = np.asarray(bk, f32)  # noqa: F841  (cancels in softmax)
    Wv = np.asarray(Wv, f32)
    bv = np.asarray(bv, f32)
    Wo = np.asarray(Wo, f32)
    bo = np.asarray(bo, f32)

    import ml_dtypes
    bf = ml_dtypes.bfloat16
    a0 = (Wq.T @ Wk / 32.0).astype(f32)         # [d, e]
    a = np.zeros((128, 128), bf)
    a[0:64, 0:64] = a0
    a[64:128, 64:128] = a0
    wvts = np.zeros((64, 2, 128), f32)
    wvts[:, 0, 0:64] = Wv.T
    wvts[:, 1, 64:128] = Wv.T
    ones64 = np.zeros((65, 2, 128), f32)
    ones64[64, 0, 0:64] = 1.0
    ones64[64, 1, 64:128] = 1.0
    # bv contributes a constant row: fold into bo
    bo_eff = bo + Wo @ np.tile(bv, HEADS)

    has_wbias = bool(np.any(bq != 0.0))
    if has_wbias:
        m = (Wk.T @ bq / 32.0).astype(f32)      # [d]
        kh = keys.reshape(B, L, HEADS, HD)
        w_all = np.einsum("blhd,d->bhl", kh, m).astype(f32)

    qh = query.reshape(B, L, HEADS, HD)
    khds = keys.reshape(B, L, HEADS, HD)
    vh = values.reshape(B, L, HEADS, HD)

    in_maps = []
    for c in range(NCORES):
        b = c // 4
        h0 = 4 * (c % 4)
        hs = list(range(h0, h0 + HPC))
        # [128, PAIRS, L]: head pair stacked on partitions (mirrors kt)
        qt = np.empty((128, PAIRS, L), bf)
        for p in range(PAIRS):
            qt[0:64, p, :] = qh[b, :, hs[2 * p], :].T
            qt[64:128, p, :] = qh[b, :, hs[2 * p + 1], :].T
        kt = np.empty((128, PAIRS, L), bf)
        for p in range(PAIRS):
            kt[0:64, p, :] = khds[b, :, hs[2 * p], :].T
            kt[64:128, p, :] = khds[b, :, hs[2 * p + 1], :].T
        v = np.empty((128, HPC, NLKT, 65), bf)
        for i in range(HPC):
            v[:, i, :, 0:64] = vh[b, :, hs[i], :].reshape(
                NLKT, 128, HD).transpose(1, 0, 2)
        v[:, :, :, 64] = 1.0
        wot = np.empty((128, PAIRS, D_MODEL), bf)
        for p in range(PAIRS):
            wot[0:64, p, :] = Wo[:, hs[2 * p] * HD:(hs[2 * p] + 1) * HD].T
            wot[64:128, p, :] = Wo[:, hs[2 * p + 1] * HD:
                                   (hs[2 * p + 1] + 1) * HD].T
        im = {
            "qt": qt,
            "kt": kt,
            "v": v,
            "a": a,
            "wvts": wvts,
            "ones64": ones64,
            "wot": wot,
        }
        if has_wbias:
            wb = np.empty((128, HPC, NLKT), f32)
            for i in range(HPC):
                wb[:, i, :] = w_all[b, hs[i]].reshape(NLKT, 128).T
            im["wb"] = wb
        in_maps.append(im)
    return in_maps, bo_eff, has_wbias


def kernel(values, keys, query, Wq, bq, Wk, bk, Wv, bv, Wo, bo,
           _trace=False):
    from concourse.bass_utils import run_bass_kernel_spmd

    in_maps, bo_eff, has_wbias = _prep_inputs(
        values, keys, query, Wq, bq, Wk, bk, Wv, bv, Wo, bo)
    nc = _get_nc(has_wbias)
    kwargs = {}
    if _trace:
        kwargs = dict(trace=True, trace_cores=[0])
    res = run_bass_kernel_spmd(nc, in_maps, core_ids=list(range(NCORES)),
                               **kwargs)
    out = np.empty((B, L, D_MODEL), np.float32)
    for b in range(B):
        acc = res.results[4 * b]["out"].astype(np.float64)
        for i in range(1, 4):
            acc += res.results[4 * b + i]["out"]
        out[b] = (acc + bo_eff).astype(np.float32)
    if _trace:
        kernel.last_exec_time_ns = res.exec_time_ns
        kernel.last_trace = res.instructions_and_trace
    return out


# revision 3
# speedup vs baseline: 1.0201x; 1.0201x over previous
"""MultiHeadAttention Trainium2 kernel: 8-core (batch, head)-sharded.

Sharding: core c handles batch c//4, heads [4*(c%4) .. 4*(c%4)+4).
Each core computes attention for its 4 heads plus its partial (row-parallel)
contribution to the output projection; host sums 4 partials per batch and
adds the bias.

Math (per batch b, head h):
  S = (Q Wq^T + bq)(K Wk^T + bk)^T / 32
    = Q A' K^T + 1 w^T + (terms constant over the softmax axis, dropped)
  with A' = Wq^T Wk / 32,  w = K (Wk^T bq) / 32   (bk cancels in softmax)
  P = softmax(S)  (no max subtraction: |S| <~ 2 for N(0,1)-scale inputs)
  O = P (V Wv^T + bv) = (P V) Wv^T + 1 bv^T
  out = sum_h O_h Wo_h^T + bo  ->  bv folds into bo on host.

v3 design: the kernel is ScalarE-bound -- 128 exp ACTIVATEs (FD=1024,
~1.2us each) are the wall; everything else hides under that stream.
  - Per lk-tile t the emission is [stE_i0, stE_i1, stO_i0, stO_i1],
    [EXP_E(t), EXP_O(t)], fillers.  The E,E prefix means EXP_E(t)'s input
    is ready the moment EXP_O(t-1) retires, so ScalarE never idles; the
    stO pair co-issues on PE row group (64,0) underneath EXP_E(t).
  - Fillers (everything that is not S^T or exp) run strictly AFTER the
    EXPs of their slot so no long-wait op ever blocks the in-order PE
    queue ahead of the critical path: U matmuls of the even head (this
    segment, lagged), U matmuls of the odd head (PREVIOUS segment, spread
    over this segment's scalar-bound phase), tail ops (1/r broadcast via
    K=2 matmul, Wv fold, normalize-STT), and output-projection pieces of
    finished lq-blocks.
  - All matmul/E dtypes bf16 (fp8 E or V injects ~3% noise into the
    signed U sums -- it does NOT average out; measured 3.7e-2 rel err).
  - Segment-boundary work (even-U drain, u eviction) is deferred into the
    next segment's filler slots; the single u PSUM accumulator ping-pongs
    even->odd->even via WAR on its DVE eviction.
"""

import sys

sys.path.insert(0, "/opt/trn_rl_repo")

import numpy as np

HEADS = 16
D_MODEL = 1024
HD = 64
B = 2
L = 2048
NCORES = 8
HPC = 4          # heads per core
PAIRS = 2        # head pairs per core
NLQB = 4         # lq blocks per core (qat projection granularity)
LQB = L // NLQB  # 512
NLKT = L // 128  # 16 lk tiles
BL = 1024        # lq block per segment

_cache = {}


def _build(has_wbias: bool):
    import concourse.bass as bass  # noqa: F401
    import concourse.tile as tile
    from concourse import bacc, mybir

    f32 = mybir.dt.float32
    f32r = mybir.dt.float32r
    bf16 = mybir.dt.bfloat16
    Exp = mybir.ActivationFunctionType.Exp
    mult = mybir.AluOpType.mult

    nc = bacc.Bacc("TRN2", target_bir_lowering=False, debug=False,
                   num_devices=NCORES)

    qt_d = nc.dram_tensor("qt", [128, PAIRS, L], bf16, kind="ExternalInput")
    kt2e_d = nc.dram_tensor("kt2e", [128, PAIRS, L], bf16,
                            kind="ExternalInput")
    kt2o_d = nc.dram_tensor("kt2o", [128, PAIRS, L], bf16,
                            kind="ExternalInput")
    v_d = nc.dram_tensor("v", [128, HPC, NLKT, 65], bf16,
                         kind="ExternalInput")
    a_d = nc.dram_tensor("a", [128, 2, 128], bf16, kind="ExternalInput")
    wvts_d = nc.dram_tensor("wvts", [64, 2, 128], f32r, kind="ExternalInput")
    ones64_d = nc.dram_tensor("ones64", [65, 2, 128], f32r,
                              kind="ExternalInput")
    wot_d = nc.dram_tensor("wot", [128, PAIRS, D_MODEL], bf16,
                           kind="ExternalInput")
    if has_wbias:
        wb_d = nc.dram_tensor("wb", [128, HPC, NLKT], f32,
                              kind="ExternalInput")
    out_d = nc.dram_tensor("out", [L, D_MODEL], f32, kind="ExternalOutput")

    NSEG = (L // BL) * PAIRS  # 4 segments: (b, p) = (0,0),(0,1),(1,0),(1,1)

    with tile.TileContext(nc) as tc:
        with (
            tc.tile_pool(name="big", bufs=1) as big,
            tc.tile_pool(name="epool", bufs=40) as epool,
            tc.tile_pool(name="small", bufs=2) as small,
            tc.tile_pool(name="stg", bufs=4) as stgp,
            tc.tile_pool(name="stp", bufs=1, space="PSUM") as stp,
            tc.tile_pool(name="up", bufs=1, space="PSUM") as up,
            tc.tile_pool(name="auxp", bufs=2, space="PSUM") as auxp,
        ):
            # ---- loads, earliest-needed first; split so the first
            # segment's compute starts after ~1MB instead of ~3MB
            a_sb = big.tile([128, 2, 128], bf16)
            nc.sync.dma_start(a_sb[:], a_d[:])
            qt_sb = big.tile([128, PAIRS, L], bf16)
            nc.sync.dma_start(qt_sb[:, 0, 0:BL], qt_d[:, 0, 0:BL])
            kt2e_sb = big.tile([128, PAIRS, L], bf16)
            nc.sync.dma_start(kt2e_sb[:, 0, :], kt2e_d[:, 0, :])
            kt2o_sb = big.tile([128, PAIRS, L], bf16)
            nc.sync.dma_start(kt2o_sb[:, 0, :], kt2o_d[:, 0, :])
            v_sb = big.tile([128, HPC, NLKT, 65], bf16)
            nc.sync.dma_start(v_sb[:, 0:1], v_d[:, 0:1])
            nc.sync.dma_start(qt_sb[:, 0, BL:L], qt_d[:, 0, BL:L])
            nc.sync.dma_start(qt_sb[:, 1, :], qt_d[:, 1, :])
            nc.sync.dma_start(v_sb[:, 1:2], v_d[:, 1:2])
            nc.sync.dma_start(kt2e_sb[:, 1, :], kt2e_d[:, 1, :])
            nc.sync.dma_start(kt2o_sb[:, 1, :], kt2o_d[:, 1, :])
            nc.sync.dma_start(v_sb[:, 2:4], v_d[:, 2:4])
            ones64_sb = big.tile([65, 2, 128], f32r)
            nc.sync.dma_start(ones64_sb[:], ones64_d[:])
            wvts_sb = big.tile([64, 2, 128], f32r)
            nc.sync.dma_start(wvts_sb[:], wvts_d[:])
            wot_sb = big.tile([128, PAIRS, D_MODEL], bf16)
            nc.sync.dma_start(wot_sb[:], wot_d[:])
            if has_wbias:
                wb_sb = big.tile([128, HPC, NLKT], f32)
                nc.sync.dma_start(wb_sb[:], wb_d[:])

            # ---- Qa^T = A'-projection of Q^T (block-diagonal A' projects
            # both heads of a pair in one K=128 matmul).  Only the two
            # blocks the first segment needs run upfront; the rest defer
            # into segment 0's filler slots so they never sit ahead of the
            # first S^T matmuls in the PE queue.
            qat2e_sb = big.tile([128, PAIRS, L], bf16)
            qat2o_sb = big.tile([128, PAIRS, L], bf16)
            _qat_dst = [qat2e_sb, qat2o_sb]

            def emit_qat(h, p, j):
                # a_sb[:, h, :] maps q -> [A'q_h ; A'q_h] (duplicated on
                # both partition halves so each lq-half S^T matmul can use
                # its own PE row group and the pair co-issues)
                sl = slice(j * LQB, (j + 1) * LQB)
                qp = auxp.tile([128, LQB], f32, tag="aux",
                               name=f"qp{h}_{p}_{j}")
                nc.tensor.matmul(qp[:], a_sb[:, h, :], qt_sb[:, p, sl],
                                 start=True, stop=True)
                nc.vector.tensor_copy(_qat_dst[h][:, p, sl], qp[:])

            for h in range(2):
                for j in range(2):
                    emit_qat(h, 0, j)
            # deferred: remaining 12 (h, p, j) pieces in seg-0 slots

            otn_sb = [big.tile([128, L], bf16, tag=f"otn{p}",
                               name=f"otn{p}") for p in range(PAIRS)]

            # ---- tail helpers -------------------------------------------
            def emit_tail_half(pv, bv, un, i):
                """1/r broadcast + Wv fold + normalize, one lq half.

                Denominators live in un[h] row 64 (the V' ones column); two
                K=1 matmuls broadcast r_even into PSUM rows 0:64 and r_odd
                into rows 64:128 (accumulating into one bank), then one DVE
                reciprocal evicts 1/r for the normalize STT.  No partition
                DMA, no [2,BL] staging."""
                csl = slice(i * LQB, (i + 1) * LQB)
                rb = auxp.tile([128, LQB], f32, tag="aux",
                               name=f"rb{bv}_{pv}_{i}")
                nc.tensor.matmul(rb[:], ones64_sb[64:65, 0, :],
                                 un[0][64:65, csl],
                                 start=True, stop=False,
                                 tile_position=(64, 0))
                nc.tensor.matmul(rb[:], ones64_sb[64:65, 1, :],
                                 un[1][64:65, csl],
                                 start=False, stop=True,
                                 tile_position=(64, 0))
                rbs = small.tile([128, LQB], f32, tag="rbs",
                                 name=f"rbs{bv}_{pv}_{i}")
                nc.vector.reciprocal_approx_fast(out=rbs[:], in_=rb[:])
                ot = auxp.tile([128, LQB], f32, tag="aux",
                               name=f"ot{bv}_{pv}_{i}")
                nc.tensor.matmul(ot[:], wvts_sb[:, 0, :],
                                 un[0][0:64, csl], start=True, stop=False)
                nc.tensor.matmul(ot[:], wvts_sb[:, 1, :],
                                 un[1][0:64, csl], start=False, stop=True)
                nc.vector.scalar_tensor_tensor(
                    out=otn_sb[pv][:, bv * BL + i * LQB:
                                   bv * BL + (i + 1) * LQB],
                    in0=ot[:], scalar=1.0, in1=rbs[:], op0=mult, op1=mult)

            def emit_proj_piece(bv, lt, scalar_evict=False):
                """Output projection for one 128-row lq tile of block bv."""
                l0 = bv * BL + lt * 128
                for nh in range(2):
                    nsl = slice(nh * 512, (nh + 1) * 512)
                    pp = auxp.tile([128, 512], f32, tag="aux",
                                   name=f"pp{bv}_{lt}_{nh}")
                    nc.tensor.matmul(pp[:], otn_sb[0][:, l0:l0 + 128],
                                     wot_sb[:, 0, nsl],
                                     start=True, stop=False)
                    nc.tensor.matmul(pp[:], otn_sb[1][:, l0:l0 + 128],
                                     wot_sb[:, 1, nsl],
                                     start=False, stop=True)
                    stg = stgp.tile([128, 512], f32, tag="stg",
                                    name=f"stg{bv}_{lt}_{nh}")
                    if scalar_evict and nh == 0:
                        # post-loop only: ScalarE is idle, let it carry
                        # half the PSUM evictions
                        nc.scalar.copy(stg[:], pp[:])
                    else:
                        nc.vector.tensor_copy(stg[:], pp[:])
                    nc.sync.dma_start(out_d[l0:l0 + 128, nsl], stg[:])

            def emit_u_mms(u, hv, chunks, e_chunks, is_first, is_last):
                """U accumulation matmuls for lk chunks (2 MMs per chunk)."""
                for tc_ in chunks:
                    for i in range(2):
                        isl = slice(i * LQB, (i + 1) * LQB)
                        nc.tensor.matmul(
                            u[:, isl], v_sb[:, hv, tc_, :],
                            e_chunks[tc_][:, isl],
                            start=(is_first and tc_ == 0),
                            stop=(is_last and tc_ == NLKT - 1))

            def finalize_u(u, bv, pv, hh):
                """Evict U rows 0:65 (row 64 = the softmax denominators)."""
                un = small.tile([65, BL], f32r, tag="un",
                                name=f"un{bv}_{pv}_{hh}", bufs=4)
                nc.vector.tensor_copy(un[:], u[0:65, :])
                return un

            # ---- main loop ----------------------------------------------
            # prev: state of segment s-1 {b, p, eE, eO, u_even, drain, un,
            #        u_odd}
            prev = None
            for si in range(NSEG):
                b, p = si // PAIRS, si % PAIRS
                eE = [epool.tile([128, BL], bf16, tag="e",
                                 name=f"eE{b}_{p}_{t}") for t in range(NLKT)]
                eO = [epool.tile([128, BL], bf16, tag="e",
                                 name=f"eO{b}_{p}_{t}") for t in range(NLKT)]
                # even-U chunk schedule: segment 0 has no odd-prev fillers,
                # so it spreads its even U over all slots (denser first
                # half keeps the PE's HAM clock warm); later segments pack
                # it into t=8..14 after the odd-prev U finishes
                if si == 0:
                    even_sched = {t: [t - 1] for t in range(1, NLKT)}
                    drain = [15]
                else:
                    even_sched = {8: [0, 1]}
                    for t_ in range(9, 15):
                        even_sched[t_] = [2 * (t_ - 8), 2 * (t_ - 8) + 1]
                    drain = [14, 15]
                u_even = None
                for t in range(NLKT):
                    # -- S^T row-packed pair matmuls: E,E then O,O
                    stE = stp.tile([128, BL], f32, tag="stE",
                                   name=f"stE{b}_{p}_{t}")
                    stO = stp.tile([128, BL], f32, tag="stO",
                                   name=f"stO{b}_{p}_{t}")
                    ksl = slice(t * 128, (t + 1) * 128)
                    for i in range(2):
                        csl = slice(i * LQB, (i + 1) * LQB)
                        qsl = slice(b * BL + i * LQB, b * BL + (i + 1) * LQB)
                        hsl = slice(64 * i, 64 * i + 64)
                        nc.tensor.matmul(stE[:, csl], kt2e_sb[hsl, p, ksl],
                                         qat2e_sb[hsl, p, qsl],
                                         start=True, stop=True,
                                         tile_position=(64 * i, 0))
                    for i in range(2):
                        csl = slice(i * LQB, (i + 1) * LQB)
                        qsl = slice(b * BL + i * LQB, b * BL + (i + 1) * LQB)
                        hsl = slice(64 * i, 64 * i + 64)
                        nc.tensor.matmul(stO[:, csl], kt2o_sb[hsl, p, ksl],
                                         qat2o_sb[hsl, p, qsl],
                                         start=True, stop=True,
                                         tile_position=(64 * i, 0))
                    # -- exp fused into the PSUM eviction (ScalarE wall)
                    biasE = (wb_sb[:, 2 * p, t:t + 1] if has_wbias else 0.0)
                    biasO = (wb_sb[:, 2 * p + 1, t:t + 1]
                             if has_wbias else 0.0)
                    nc.scalar.activation(eE[t][:], stE[:], Exp, bias=biasE)
                    nc.scalar.activation(eO[t][:], stO[:], Exp, bias=biasO)
                    # -- fillers (strictly after the EXPs: nothing here may
                    # ever sit ahead of the next S^T in the PE queue)
                    if t == 0:
                        if prev is not None:
                            # drain prev even-U tail chunks + evict
                            emit_u_mms(prev["u_even"], 2 * prev["p"],
                                       prev["drain"], prev["eE"],
                                       False, True)
                            prev["un"] = {0: finalize_u(
                                prev["u_even"], prev["b"], prev["p"], 0)}
                    else:
                        if prev is not None and 1 <= t <= 7:
                            if t == 1:
                                prev["u_odd"] = up.tile(
                                    [65, BL], f32, tag="u",
                                    name=f"uO{prev['b']}_{prev['p']}")
                            emit_u_mms(prev["u_odd"], 2 * prev["p"] + 1,
                                       [2 * (t - 1), 2 * t - 1],
                                       prev["eO"], True, False)
                        if prev is not None and t == 8:
                            emit_u_mms(prev["u_odd"], 2 * prev["p"] + 1,
                                       [14, 15], prev["eO"], False, True)
                            prev["un"][1] = finalize_u(
                                prev["u_odd"], prev["b"], prev["p"], 1)
                        if t in even_sched:
                            if u_even is None:
                                u_even = up.tile([65, BL], f32, tag="u",
                                                 name=f"uE{b}_{p}")
                            emit_u_mms(u_even, 2 * p, even_sched[t], eE,
                                       even_sched[t][0] == 0, False)
                        if si == 0 and 1 <= t <= 6:
                            for args in [
                                [(0, 0, 2), (1, 0, 2)],
                                [(0, 0, 3), (1, 0, 3)],
                                [(0, 1, 0), (1, 1, 0)],
                                [(0, 1, 1), (1, 1, 1)],
                                [(0, 1, 2), (1, 1, 2)],
                                [(0, 1, 3), (1, 1, 3)],
                            ][t - 1]:
                                emit_qat(*args)
                        if prev is not None:
                            if t == 11:
                                emit_tail_half(prev["p"], prev["b"],
                                               prev["un"], 0)
                            elif t == 12:
                                emit_tail_half(prev["p"], prev["b"],
                                               prev["un"], 1)
                            elif t >= 13 and p == 0 and b >= 1:
                                # proj of finished block b-1: lt 0..5 here,
                                # lt 6,7 spill into the next segment
                                for lt in range(2 * (t - 13),
                                                2 * (t - 13) + 2):
                                    emit_proj_piece(b - 1, lt)
                            if p == 1 and b >= 1 and t in (5, 6):
                                emit_proj_piece(b - 1, 1 + t)
                        if si == NSEG - 1 and t == 15:
                            # last segment: drain + evict even U now so
                            # the post-loop odd burst starts immediately
                            emit_u_mms(u_even, 2 * p, drain, eE,
                                       False, True)
                            un_last = {0: finalize_u(u_even, b, p, 0)}
                prev = {"b": b, "p": p, "eE": eE, "eO": eO,
                        "u_even": u_even, "drain": drain, "un": None,
                        "u_odd": None}

            # ---- post-loop: last segment's odd head + tails + proj(1).
            # The odd-U burst runs i=0 then i=1; each un half evicts
            # behind the other half's matmuls so the tail chain never
            # stalls the PE queue.
            prev["un"] = un_last
            u_odd = up.tile([65, BL], f32, tag="u", name="uO_last")
            unO = None
            for i in range(2):
                isl = slice(i * LQB, (i + 1) * LQB)
                for tc_ in range(NLKT):
                    nc.tensor.matmul(u_odd[:, isl],
                                     v_sb[:, 2 * prev["p"] + 1, tc_, :],
                                     prev["eO"][tc_][:, isl],
                                     start=(tc_ == 0), stop=(tc_ == 15))
                if unO is None:
                    unO = small.tile([65, BL], f32r, tag="un",
                                     name="unO_last", bufs=4)
                nc.vector.tensor_copy(unO[:, isl], u_odd[0:65, isl])
            prev["un"][1] = unO
            emit_tail_half(prev["p"], prev["b"], prev["un"], 0)
            for lt in range(0, 4):
                emit_proj_piece(1, lt, scalar_evict=True)
            emit_tail_half(prev["p"], prev["b"], prev["un"], 1)
            for lt in range(4, 8):
                emit_proj_piece(1, lt, scalar_evict=True)
    nc.compile()
    return nc


def _get_nc(has_wbias: bool):
    key = ("nc", has_wbias)
    if key not in _cache:
        _cache[key] = _build(has_wbias)
    return _cache[key]


def _prep_inputs(values, keys, query, Wq, bq, Wk, bk, Wv, bv, Wo, bo):
    """Host-side shard/layout prep. Returns (in_maps, bo_eff, has_wbias)."""
    f32 = np.float32
    values = np.asarray(values, f32)
    keys = np.asarray(keys, f32)
    query = np.asarray(query, f32)
    Wq = np.asarray(Wq, f32)
    bq = np.asarray(bq, f32)
    Wk = np.asarray(Wk, f32)
    bk = np.asarray(bk, f32)  # noqa: F841  (cancels in softmax)
    Wv = np.asarray(Wv, f32)
    bv = np.asarray(bv, f32)
    Wo = np.asarray(Wo, f32)
    bo = np.asarray(bo, f32)

    import ml_dtypes
    bf = ml_dtypes.bfloat16
    a0 = (Wq.T @ Wk / 32.0).astype(f32)         # [d, e]
    # a[:, 0, :]: q -> [A'q_E ; A'q_E]   a[:, 1, :]: q -> [A'q_O ; A'q_O]
    # (lhsT convention: out[m] = sum_k a[k, m] * q[k])
    a = np.zeros((128, 2, 128), bf)
    a[0:64, 0, 0:64] = a0
    a[0:64, 0, 64:128] = a0
    a[64:128, 1, 0:64] = a0
    a[64:128, 1, 64:128] = a0
    wvts = np.zeros((64, 2, 128), f32)
    wvts[:, 0, 0:64] = Wv.T
    wvts[:, 1, 64:128] = Wv.T
    ones64 = np.zeros((65, 2, 128), f32)
    ones64[64, 0, 0:64] = 1.0
    ones64[64, 1, 64:128] = 1.0
    # bv contributes a constant row: fold into bo
    bo_eff = bo + Wo @ np.tile(bv, HEADS)

    has_wbias = bool(np.any(bq != 0.0))
    if has_wbias:
        m = (Wk.T @ bq / 32.0).astype(f32)      # [d]
        kh = keys.reshape(B, L, HEADS, HD)
        w_all = np.einsum("blhd,d->bhl", kh, m).astype(f32)

    qh = query.reshape(B, L, HEADS, HD)
    khds = keys.reshape(B, L, HEADS, HD)
    vh = values.reshape(B, L, HEADS, HD)

    in_maps = []
    for c in range(NCORES):
        b = c // 4
        h0 = 4 * (c % 4)
        hs = list(range(h0, h0 + HPC))
        # [128, PAIRS, L]: head pair stacked on partitions (mirrors kt)
        qt = np.empty((128, PAIRS, L), bf)
        for p in range(PAIRS):
            qt[0:64, p, :] = qh[b, :, hs[2 * p], :].T
            qt[64:128, p, :] = qh[b, :, hs[2 * p + 1], :].T
        # per-head K^T duplicated on both partition halves (row-group
        # co-issue of the two lq-half S^T matmuls)
        kt2e = np.empty((128, PAIRS, L), bf)
        kt2o = np.empty((128, PAIRS, L), bf)
        for p in range(PAIRS):
            ke = khds[b, :, hs[2 * p], :].T
            ko = khds[b, :, hs[2 * p + 1], :].T
            kt2e[0:64, p, :] = ke
            kt2e[64:128, p, :] = ke
            kt2o[0:64, p, :] = ko
            kt2o[64:128, p, :] = ko
        v = np.empty((128, HPC, NLKT, 65), bf)
        for i in range(HPC):
            v[:, i, :, 0:64] = vh[b, :, hs[i], :].reshape(
                NLKT, 128, HD).transpose(1, 0, 2)
        v[:, :, :, 64] = 1.0
        wot = np.empty((128, PAIRS, D_MODEL), bf)
        for p in range(PAIRS):
            wot[0:64, p, :] = Wo[:, hs[2 * p] * HD:(hs[2 * p] + 1) * HD].T
            wot[64:128, p, :] = Wo[:, hs[2 * p + 1] * HD:
                                   (hs[2 * p + 1] + 1) * HD].T
        im = {
            "qt": qt,
            "kt2e": kt2e,
            "kt2o": kt2o,
            "v": v,
            "a": a,
            "wvts": wvts,
            "ones64": ones64,
            "wot": wot,
        }
        if has_wbias:
            wb = np.empty((128, HPC, NLKT), f32)
            for i in range(HPC):
                wb[:, i, :] = w_all[b, hs[i]].reshape(NLKT, 128).T
            im["wb"] = wb
        in_maps.append(im)
    return in_maps, bo_eff, has_wbias


def kernel(values, keys, query, Wq, bq, Wk, bk, Wv, bv, Wo, bo,
           _trace=False):
    from concourse.bass_utils import run_bass_kernel_spmd

    in_maps, bo_eff, has_wbias = _prep_inputs(
        values, keys, query, Wq, bq, Wk, bk, Wv, bv, Wo, bo)
    nc = _get_nc(has_wbias)
    kwargs = {}
    if _trace:
        kwargs = dict(trace=True, trace_cores=[0])
    res = run_bass_kernel_spmd(nc, in_maps, core_ids=list(range(NCORES)),
                               **kwargs)
    out = np.empty((B, L, D_MODEL), np.float32)
    for b in range(B):
        acc = res.results[4 * b]["out"].astype(np.float64)
        for i in range(1, 4):
            acc += res.results[4 * b + i]["out"]
        out[b] = (acc + bo_eff).astype(np.float32)
    if _trace:
        kernel.last_exec_time_ns = res.exec_time_ns
        kernel.last_trace = res.instructions_and_trace
    return out


# revision 4
# speedup vs baseline: 1.0304x; 1.0101x over previous
"""MultiHeadAttention Trainium2 kernel: 8-core (batch, head)-sharded.

Sharding: core c handles batch c//4, heads [4*(c%4) .. 4*(c%4)+4).
Each core computes attention for its 4 heads plus its partial (row-parallel)
contribution to the output projection; host sums 4 partials per batch and
adds the bias.

Math (per batch b, head h):
  S = (Q Wq^T + bq)(K Wk^T + bk)^T / 32
    = Q A' K^T + 1 w^T + (terms constant over the softmax axis, dropped)
  with A' = Wq^T Wk / 32,  w = K (Wk^T bq) / 32   (bk cancels in softmax)
  P = softmax(S)  (no max subtraction: |S| <~ 2 for N(0,1)-scale inputs)
  O = P (V Wv^T + bv) = (P V) Wv^T + 1 bv^T
  out = sum_h O_h Wo_h^T + bo  ->  bv folds into bo on host.

v3 design: the kernel is ScalarE-bound -- 128 exp ACTIVATEs (FD=1024,
~1.2us each) are the wall; everything else hides under that stream.
  - Per lk-tile t the emission is [stE_i0, stE_i1, stO_i0, stO_i1],
    [EXP_E(t), EXP_O(t)], fillers.  The E,E prefix means EXP_E(t)'s input
    is ready the moment EXP_O(t-1) retires, so ScalarE never idles; the
    stO pair co-issues on PE row group (64,0) underneath EXP_E(t).
  - Fillers (everything that is not S^T or exp) run strictly AFTER the
    EXPs of their slot so no long-wait op ever blocks the in-order PE
    queue ahead of the critical path: U matmuls of the even head (this
    segment, lagged), U matmuls of the odd head (PREVIOUS segment, spread
    over this segment's scalar-bound phase), tail ops (1/r broadcast via
    K=2 matmul, Wv fold, normalize-STT), and output-projection pieces of
    finished lq-blocks.
  - All matmul/E dtypes bf16 (fp8 E or V injects ~3% noise into the
    signed U sums -- it does NOT average out; measured 3.7e-2 rel err).
  - Segment-boundary work (even-U drain, u eviction) is deferred into the
    next segment's filler slots; the single u PSUM accumulator ping-pongs
    even->odd->even via WAR on its DVE eviction.
"""

import sys

sys.path.insert(0, "/opt/trn_rl_repo")

import numpy as np

HEADS = 16
D_MODEL = 1024
HD = 64
B = 2
L = 2048
NCORES = 8
HPC = 4          # heads per core
PAIRS = 2        # head pairs per core
NLQB = 4         # lq blocks per core (qat projection granularity)
LQB = L // NLQB  # 512
NLKT = L // 128  # 16 lk tiles
BL = 1024        # lq block per segment

_cache = {}


def _build(has_wbias: bool):
    import concourse.bass as bass  # noqa: F401
    import concourse.tile as tile
    from concourse import bacc, mybir

    f32 = mybir.dt.float32
    f32r = mybir.dt.float32r
    bf16 = mybir.dt.bfloat16
    Exp = mybir.ActivationFunctionType.Exp
    mult = mybir.AluOpType.mult

    nc = bacc.Bacc("TRN2", target_bir_lowering=False, debug=False,
                   num_devices=NCORES)

    qt_d = nc.dram_tensor("qt", [128, PAIRS, L], bf16, kind="ExternalInput")
    kt2e_d = nc.dram_tensor("kt2e", [128, PAIRS, L], bf16,
                            kind="ExternalInput")
    kt2o_d = nc.dram_tensor("kt2o", [128, PAIRS, L], bf16,
                            kind="ExternalInput")
    v_d = nc.dram_tensor("v", [128, HPC, NLKT, 65], bf16,
                         kind="ExternalInput")
    a_d = nc.dram_tensor("a", [128, 2, 128], bf16, kind="ExternalInput")
    wvts_d = nc.dram_tensor("wvts", [64, 2, 128], f32r, kind="ExternalInput")
    ones64_d = nc.dram_tensor("ones64", [65, 2, 128], f32r,
                              kind="ExternalInput")
    wot_d = nc.dram_tensor("wot", [128, PAIRS, D_MODEL], bf16,
                           kind="ExternalInput")
    if has_wbias:
        wb_d = nc.dram_tensor("wb", [128, HPC, NLKT], f32,
                              kind="ExternalInput")
    out_d = nc.dram_tensor("out", [L, D_MODEL], bf16,
                           kind="ExternalOutput")

    NSEG = (L // BL) * PAIRS  # 4 segments: (b, p) = (0,0),(0,1),(1,0),(1,1)

    with tile.TileContext(nc) as tc:
        with (
            tc.tile_pool(name="big", bufs=1) as big,
            tc.tile_pool(name="epool", bufs=40) as epool,
            tc.tile_pool(name="small", bufs=2) as small,
            tc.tile_pool(name="stg", bufs=4) as stgp,
            tc.tile_pool(name="stp", bufs=1, space="PSUM") as stp,
            tc.tile_pool(name="up", bufs=1, space="PSUM") as up,
            tc.tile_pool(name="auxp", bufs=2, space="PSUM") as auxp,
        ):
            # ---- loads, earliest-needed first; split so the first
            # segment's compute starts after ~1MB instead of ~3MB
            a_sb = big.tile([128, 2, 128], bf16)
            nc.sync.dma_start(a_sb[:], a_d[:])
            qt_sb = big.tile([128, PAIRS, L], bf16)
            nc.sync.dma_start(qt_sb[:, 0, 0:LQB], qt_d[:, 0, 0:LQB])
            nc.sync.dma_start(qt_sb[:, 0, LQB:BL], qt_d[:, 0, LQB:BL])
            kt2e_sb = big.tile([128, PAIRS, L], bf16)
            nc.sync.dma_start(kt2e_sb[:, 0, :], kt2e_d[:, 0, :])
            kt2o_sb = big.tile([128, PAIRS, L], bf16)
            nc.sync.dma_start(kt2o_sb[:, 0, :], kt2o_d[:, 0, :])
            v_sb = big.tile([128, HPC, NLKT, 65], bf16)
            nc.sync.dma_start(v_sb[:, 0:1], v_d[:, 0:1])
            nc.sync.dma_start(qt_sb[:, 0, BL:L], qt_d[:, 0, BL:L])
            nc.sync.dma_start(qt_sb[:, 1, :], qt_d[:, 1, :])
            nc.sync.dma_start(v_sb[:, 1:2], v_d[:, 1:2])
            nc.sync.dma_start(kt2e_sb[:, 1, :], kt2e_d[:, 1, :])
            nc.sync.dma_start(kt2o_sb[:, 1, :], kt2o_d[:, 1, :])
            nc.sync.dma_start(v_sb[:, 2:4], v_d[:, 2:4])
            ones64_sb = big.tile([65, 2, 128], f32r)
            nc.sync.dma_start(ones64_sb[:], ones64_d[:])
            wvts_sb = big.tile([64, 2, 128], f32r)
            nc.sync.dma_start(wvts_sb[:], wvts_d[:])
            wot_sb = big.tile([128, PAIRS, D_MODEL], bf16)
            nc.sync.dma_start(wot_sb[:], wot_d[:])
            if has_wbias:
                wb_sb = big.tile([128, HPC, NLKT], f32)
                nc.sync.dma_start(wb_sb[:], wb_d[:])

            # ---- Qa^T = A'-projection of Q^T (block-diagonal A' projects
            # both heads of a pair in one K=128 matmul).  Only the two
            # blocks the first segment needs run upfront; the rest defer
            # into segment 0's filler slots so they never sit ahead of the
            # first S^T matmuls in the PE queue.
            qat2e_sb = big.tile([128, PAIRS, L], bf16)
            qat2o_sb = big.tile([128, PAIRS, L], bf16)
            _qat_dst = [qat2e_sb, qat2o_sb]

            def emit_qat(h, p, j):
                # a_sb[:, h, :] maps q -> [A'q_h ; A'q_h] (duplicated on
                # both partition halves so each lq-half S^T matmul can use
                # its own PE row group and the pair co-issues)
                sl = slice(j * LQB, (j + 1) * LQB)
                qp = auxp.tile([128, LQB], f32, tag="aux",
                               name=f"qp{h}_{p}_{j}")
                nc.tensor.matmul(qp[:], a_sb[:, h, :], qt_sb[:, p, sl],
                                 start=True, stop=True)
                nc.vector.tensor_copy(_qat_dst[h][:, p, sl], qp[:])

            for j in range(2):
                for h in range(2):
                    emit_qat(h, 0, j)
            # deferred: remaining 12 (h, p, j) pieces in seg-0 slots

            otn_sb = [big.tile([128, L], bf16, tag=f"otn{p}",
                               name=f"otn{p}") for p in range(PAIRS)]

            # ---- tail helpers -------------------------------------------
            def emit_tail_half(pv, bv, un, i):
                """1/r broadcast + Wv fold + normalize, one lq half.

                Denominators live in un[h] row 64 (the V' ones column); two
                K=1 matmuls broadcast r_even into PSUM rows 0:64 and r_odd
                into rows 64:128 (accumulating into one bank), then one DVE
                reciprocal evicts 1/r for the normalize STT.  No partition
                DMA, no [2,BL] staging."""
                csl = slice(i * LQB, (i + 1) * LQB)
                rb = auxp.tile([128, LQB], f32, tag="aux",
                               name=f"rb{bv}_{pv}_{i}")
                nc.tensor.matmul(rb[:], ones64_sb[64:65, 0, :],
                                 un[0][64:65, csl],
                                 start=True, stop=False,
                                 tile_position=(64, 0))
                nc.tensor.matmul(rb[:], ones64_sb[64:65, 1, :],
                                 un[1][64:65, csl],
                                 start=False, stop=True,
                                 tile_position=(64, 0))
                rbs = small.tile([128, LQB], f32, tag="rbs",
                                 name=f"rbs{bv}_{pv}_{i}")
                nc.vector.reciprocal_approx_fast(out=rbs[:], in_=rb[:])
                ot = auxp.tile([128, LQB], f32, tag="aux",
                               name=f"ot{bv}_{pv}_{i}")
                nc.tensor.matmul(ot[:], wvts_sb[:, 0, :],
                                 un[0][0:64, csl], start=True, stop=False)
                nc.tensor.matmul(ot[:], wvts_sb[:, 1, :],
                                 un[1][0:64, csl], start=False, stop=True)
                nc.vector.scalar_tensor_tensor(
                    out=otn_sb[pv][:, bv * BL + i * LQB:
                                   bv * BL + (i + 1) * LQB],
                    in0=ot[:], scalar=1.0, in1=rbs[:], op0=mult, op1=mult)

            def emit_proj_piece(bv, lt, scalar_evict=False):
                """Output projection for one 128-row lq tile of block bv."""
                l0 = bv * BL + lt * 128
                for nh in range(2):
                    nsl = slice(nh * 512, (nh + 1) * 512)
                    pp = auxp.tile([128, 512], f32, tag="aux",
                                   name=f"pp{bv}_{lt}_{nh}")
                    nc.tensor.matmul(pp[:], otn_sb[0][:, l0:l0 + 128],
                                     wot_sb[:, 0, nsl],
                                     start=True, stop=False)
                    nc.tensor.matmul(pp[:], otn_sb[1][:, l0:l0 + 128],
                                     wot_sb[:, 1, nsl],
                                     start=False, stop=True)
                    stg = stgp.tile([128, 512], bf16, tag="stg",
                                    name=f"stg{bv}_{lt}_{nh}")
                    if scalar_evict and nh == 0:
                        # post-loop only: ScalarE is idle, let it carry
                        # half the PSUM evictions
                        nc.scalar.copy(stg[:], pp[:])
                    else:
                        nc.vector.tensor_copy(stg[:], pp[:])
                    nc.sync.dma_start(out_d[l0:l0 + 128, nsl], stg[:])

            def emit_u_mms(u, hv, chunks, e_chunks, is_first, is_last):
                """U accumulation matmuls for lk chunks (2 MMs per chunk)."""
                for tc_ in chunks:
                    for i in range(2):
                        isl = slice(i * LQB, (i + 1) * LQB)
                        nc.tensor.matmul(
                            u[:, isl], v_sb[:, hv, tc_, :],
                            e_chunks[tc_][:, isl],
                            start=(is_first and tc_ == 0),
                            stop=(is_last and tc_ == NLKT - 1))

            def finalize_u(u, bv, pv, hh):
                """Evict U rows 0:65 (row 64 = the softmax denominators)."""
                un = small.tile([65, BL], f32r, tag="un",
                                name=f"un{bv}_{pv}_{hh}", bufs=4)
                nc.vector.tensor_copy(un[:], u[0:65, :])
                return un

            # ---- main loop ----------------------------------------------
            # prev: state of segment s-1 {b, p, eE, eO, u_even, drain, un,
            #        u_odd}
            prev = None
            for si in range(NSEG):
                b, p = si // PAIRS, si % PAIRS
                eE = [epool.tile([128, BL], bf16, tag="e",
                                 name=f"eE{b}_{p}_{t}") for t in range(NLKT)]
                eO = [epool.tile([128, BL], bf16, tag="e",
                                 name=f"eO{b}_{p}_{t}") for t in range(NLKT)]
                # even-U chunk schedule: segment 0 has no odd-prev fillers,
                # so it spreads its even U over all slots (denser first
                # half keeps the PE's HAM clock warm); later segments pack
                # it into t=8..14 after the odd-prev U finishes
                if si == 0:
                    even_sched = {t: [t - 1] for t in range(1, NLKT)}
                    drain = [15]
                else:
                    even_sched = {8: [0, 1]}
                    for t_ in range(9, 15):
                        even_sched[t_] = [2 * (t_ - 8), 2 * (t_ - 8) + 1]
                    drain = [14, 15]
                u_even = None
                for t in range(NLKT):
                    # -- S^T row-packed pair matmuls: E,E then O,O
                    stE = stp.tile([128, BL], f32, tag="stE",
                                   name=f"stE{b}_{p}_{t}")
                    stO = stp.tile([128, BL], f32, tag="stO",
                                   name=f"stO{b}_{p}_{t}")
                    ksl = slice(t * 128, (t + 1) * 128)
                    for i in range(2):
                        csl = slice(i * LQB, (i + 1) * LQB)
                        qsl = slice(b * BL + i * LQB, b * BL + (i + 1) * LQB)
                        hsl = slice(64 * i, 64 * i + 64)
                        nc.tensor.matmul(stE[:, csl], kt2e_sb[hsl, p, ksl],
                                         qat2e_sb[hsl, p, qsl],
                                         start=True, stop=True,
                                         tile_position=(64 * i, 0))
                    for i in range(2):
                        csl = slice(i * LQB, (i + 1) * LQB)
                        qsl = slice(b * BL + i * LQB, b * BL + (i + 1) * LQB)
                        hsl = slice(64 * i, 64 * i + 64)
                        nc.tensor.matmul(stO[:, csl], kt2o_sb[hsl, p, ksl],
                                         qat2o_sb[hsl, p, qsl],
                                         start=True, stop=True,
                                         tile_position=(64 * i, 0))
                    # -- exp fused into the PSUM eviction (ScalarE wall)
                    biasE = (wb_sb[:, 2 * p, t:t + 1] if has_wbias else 0.0)
                    biasO = (wb_sb[:, 2 * p + 1, t:t + 1]
                             if has_wbias else 0.0)
                    nc.scalar.activation(eE[t][:], stE[:], Exp, bias=biasE)
                    nc.scalar.activation(eO[t][:], stO[:], Exp, bias=biasO)
                    # -- fillers (strictly after the EXPs: nothing here may
                    # ever sit ahead of the next S^T in the PE queue)
                    if t == 0:
                        if prev is not None:
                            # drain prev even-U tail chunks + evict
                            emit_u_mms(prev["u_even"], 2 * prev["p"],
                                       prev["drain"], prev["eE"],
                                       False, True)
                            prev["un"] = {0: finalize_u(
                                prev["u_even"], prev["b"], prev["p"], 0)}
                    else:
                        if prev is not None and 1 <= t <= 7:
                            if t == 1:
                                prev["u_odd"] = up.tile(
                                    [65, BL], f32, tag="u",
                                    name=f"uO{prev['b']}_{prev['p']}")
                            emit_u_mms(prev["u_odd"], 2 * prev["p"] + 1,
                                       [2 * (t - 1), 2 * t - 1],
                                       prev["eO"], True, False)
                        if prev is not None and t == 8:
                            emit_u_mms(prev["u_odd"], 2 * prev["p"] + 1,
                                       [14, 15], prev["eO"], False, True)
                            prev["un"][1] = finalize_u(
                                prev["u_odd"], prev["b"], prev["p"], 1)
                        if t in even_sched:
                            if u_even is None:
                                u_even = up.tile([65, BL], f32, tag="u",
                                                 name=f"uE{b}_{p}")
                            emit_u_mms(u_even, 2 * p, even_sched[t], eE,
                                       even_sched[t][0] == 0, False)
                        if si == 0 and 1 <= t <= 6:
                            for args in [
                                [(0, 0, 2), (1, 0, 2)],
                                [(0, 0, 3), (1, 0, 3)],
                                [(0, 1, 0), (1, 1, 0)],
                                [(0, 1, 1), (1, 1, 1)],
                                [(0, 1, 2), (1, 1, 2)],
                                [(0, 1, 3), (1, 1, 3)],
                            ][t - 1]:
                                emit_qat(*args)
                        if prev is not None:
                            if t == 11:
                                emit_tail_half(prev["p"], prev["b"],
                                               prev["un"], 0)
                            elif t == 12:
                                emit_tail_half(prev["p"], prev["b"],
                                               prev["un"], 1)
                            elif t >= 13 and p == 0 and b >= 1:
                                # proj of finished block b-1: lt 0..5 here,
                                # lt 6,7 spill into the next segment
                                for lt in range(2 * (t - 13),
                                                2 * (t - 13) + 2):
                                    emit_proj_piece(b - 1, lt)
                            if p == 1 and b >= 1 and t in (5, 6):
                                emit_proj_piece(b - 1, 1 + t)
                        if si == NSEG - 1 and t == 15:
                            # last segment: drain + evict even U now so
                            # the post-loop odd burst starts immediately
                            emit_u_mms(u_even, 2 * p, drain, eE,
                                       False, True)
                            un_last = {0: finalize_u(u_even, b, p, 0)}
                prev = {"b": b, "p": p, "eE": eE, "eO": eO,
                        "u_even": u_even, "drain": drain, "un": None,
                        "u_odd": None}

            # ---- post-loop: last segment's odd head + tails + proj(1).
            # The odd-U burst runs i=0 then i=1; each un half evicts
            # behind the other half's matmuls so the tail chain never
            # stalls the PE queue.
            prev["un"] = un_last
            u_odd = up.tile([65, BL], f32, tag="u", name="uO_last")
            unO = None
            for i in range(2):
                isl = slice(i * LQB, (i + 1) * LQB)
                for tc_ in range(NLKT):
                    nc.tensor.matmul(u_odd[:, isl],
                                     v_sb[:, 2 * prev["p"] + 1, tc_, :],
                                     prev["eO"][tc_][:, isl],
                                     start=(tc_ == 0), stop=(tc_ == 15))
                if unO is None:
                    unO = small.tile([65, BL], f32r, tag="un",
                                     name="unO_last", bufs=4)
                nc.vector.tensor_copy(unO[:, isl], u_odd[0:65, isl])
            prev["un"][1] = unO
            emit_tail_half(prev["p"], prev["b"], prev["un"], 0)
            for lt in range(0, 4):
                emit_proj_piece(1, lt, scalar_evict=True)
            emit_tail_half(prev["p"], prev["b"], prev["un"], 1)
            for lt in range(4, 8):
                emit_proj_piece(1, lt, scalar_evict=True)
    nc.compile()
    return nc


def _get_nc(has_wbias: bool):
    key = ("nc", has_wbias)
    if key not in _cache:
        _cache[key] = _build(has_wbias)
    return _cache[key]


def _prep_inputs(values, keys, query, Wq, bq, Wk, bk, Wv, bv, Wo, bo):
    """Host-side shard/layout prep. Returns (in_maps, bo_eff, has_wbias)."""
    f32 = np.float32
    values = np.asarray(values, f32)
    keys = np.asarray(keys, f32)
    query = np.asarray(query, f32)
    Wq = np.asarray(Wq, f32)
    bq = np.asarray(bq, f32)
    Wk = np.asarray(Wk, f32)
    bk = np.asarray(bk, f32)  # noqa: F841  (cancels in softmax)
    Wv = np.asarray(Wv, f32)
    bv = np.asarray(bv, f32)
    Wo = np.asarray(Wo, f32)
    bo = np.asarray(bo, f32)

    import ml_dtypes
    bf = ml_dtypes.bfloat16
    a0 = (Wq.T @ Wk / 32.0).astype(f32)         # [d, e]
    # a[:, 0, :]: q -> [A'q_E ; A'q_E]   a[:, 1, :]: q -> [A'q_O ; A'q_O]
    # (lhsT convention: out[m] = sum_k a[k, m] * q[k])
    a = np.zeros((128, 2, 128), bf)
    a[0:64, 0, 0:64] = a0
    a[0:64, 0, 64:128] = a0
    a[64:128, 1, 0:64] = a0
    a[64:128, 1, 64:128] = a0
    wvts = np.zeros((64, 2, 128), f32)
    wvts[:, 0, 0:64] = Wv.T
    wvts[:, 1, 64:128] = Wv.T
    ones64 = np.zeros((65, 2, 128), f32)
    ones64[64, 0, 0:64] = 1.0
    ones64[64, 1, 64:128] = 1.0
    # bv contributes a constant row: fold into bo
    bo_eff = bo + Wo @ np.tile(bv, HEADS)

    has_wbias = bool(np.any(bq != 0.0))
    if has_wbias:
        m = (Wk.T @ bq / 32.0).astype(f32)      # [d]
        kh = keys.reshape(B, L, HEADS, HD)
        w_all = np.einsum("blhd,d->bhl", kh, m).astype(f32)

    qh = query.reshape(B, L, HEADS, HD)
    khds = keys.reshape(B, L, HEADS, HD)
    vh = values.reshape(B, L, HEADS, HD)

    in_maps = []
    for c in range(NCORES):
        b = c // 4
        h0 = 4 * (c % 4)
        hs = list(range(h0, h0 + HPC))
        # [128, PAIRS, L]: head pair stacked on partitions (mirrors kt)
        qt = np.empty((128, PAIRS, L), bf)
        for p in range(PAIRS):
            qt[0:64, p, :] = qh[b, :, hs[2 * p], :].T
            qt[64:128, p, :] = qh[b, :, hs[2 * p + 1], :].T
        # per-head K^T duplicated on both partition halves (row-group
        # co-issue of the two lq-half S^T matmuls)
        kt2e = np.empty((128, PAIRS, L), bf)
        kt2o = np.empty((128, PAIRS, L), bf)
        for p in range(PAIRS):
            ke = khds[b, :, hs[2 * p], :].T
            ko = khds[b, :, hs[2 * p + 1], :].T
            kt2e[0:64, p, :] = ke
            kt2e[64:128, p, :] = ke
            kt2o[0:64, p, :] = ko
            kt2o[64:128, p, :] = ko
        v = np.empty((128, HPC, NLKT, 65), bf)
        for i in range(HPC):
            v[:, i, :, 0:64] = vh[b, :, hs[i], :].reshape(
                NLKT, 128, HD).transpose(1, 0, 2)
        v[:, :, :, 64] = 1.0
        wot = np.empty((128, PAIRS, D_MODEL), bf)
        for p in range(PAIRS):
            wot[0:64, p, :] = Wo[:, hs[2 * p] * HD:(hs[2 * p] + 1) * HD].T
            wot[64:128, p, :] = Wo[:, hs[2 * p + 1] * HD:
                                   (hs[2 * p + 1] + 1) * HD].T
        im = {
            "qt": qt,
            "kt2e": kt2e,
            "kt2o": kt2o,
            "v": v,
            "a": a,
            "wvts": wvts,
            "ones64": ones64,
            "wot": wot,
        }
        if has_wbias:
            wb = np.empty((128, HPC, NLKT), f32)
            for i in range(HPC):
                wb[:, i, :] = w_all[b, hs[i]].reshape(NLKT, 128).T
            im["wb"] = wb
        in_maps.append(im)
    return in_maps, bo_eff, has_wbias


def kernel(values, keys, query, Wq, bq, Wk, bk, Wv, bv, Wo, bo,
           _trace=False):
    from concourse.bass_utils import run_bass_kernel_spmd

    in_maps, bo_eff, has_wbias = _prep_inputs(
        values, keys, query, Wq, bq, Wk, bk, Wv, bv, Wo, bo)
    nc = _get_nc(has_wbias)
    kwargs = {}
    if _trace:
        kwargs = dict(trace=True, trace_cores=[0])
    res = run_bass_kernel_spmd(nc, in_maps, core_ids=list(range(NCORES)),
                               **kwargs)
    out = np.empty((B, L, D_MODEL), np.float32)
    for b in range(B):
        acc = res.results[4 * b]["out"].astype(np.float64)
        for i in range(1, 4):
            acc += res.results[4 * b + i]["out"]
        out[b] = (acc + bo_eff).astype(np.float32)
    if _trace:
        kernel.last_exec_time_ns = res.exec_time_ns
        kernel.last_trace = res.instructions_and_trace
    return out


# revision 5
# speedup vs baseline: 1.0377x; 1.0070x over previous
"""MultiHeadAttention Trainium2 kernel: 8-core (batch, head)-sharded.

Sharding: core c handles batch c//4, heads [4*(c%4) .. 4*(c%4)+4).
Each core computes attention for its 4 heads plus its partial (row-parallel)
contribution to the output projection; host sums 4 partials per batch (sent
as bf16, upcast on host) and adds the bias.

Math (per batch b, head h):
  S = (Q Wq^T + bq)(K Wk^T + bk)^T / 32
    = Q A' K^T + 1 w^T + (terms constant over the softmax axis, dropped)
  with A' = Wq^T Wk / 32,  w = K (Wk^T bq) / 32   (bk cancels in softmax)
  P = softmax(S)  (no max subtraction: |S| <~ 2 for N(0,1)-scale inputs)
  O = P (V Wv^T + bv) = (P V) Wv^T + 1 bv^T
  out = sum_h O_h Wo_h^T + bo  ->  bv folds into bo on host.

Design: the kernel is ScalarE-bound -- 128 exp ACTIVATEs (FD=1024, ~1.05us
each) are the wall; everything else hides under that stream.
  - kt and Qa^T are host-duplicated onto both partition halves (kt2e/kt2o,
    qat2e/qat2o, with split A'-projection stationaries), so each head's two
    lq-half S^T matmuls run on different PE row groups (tile positions
    (0,0)/(64,0)) and co-issue ~220ns apart: EXP_E(t)'s input is ready the
    moment EXP_O(t-1) retires and ScalarE stays ~95% saturated.
  - Fillers (everything that is not S^T or exp) are emitted strictly AFTER
    each slot's ACTs so no long-wait op ever blocks the in-order PE queue
    ahead of the critical path: U matmuls of the even head (this segment,
    lagged), U matmuls of the odd head (PREVIOUS segment, spread over this
    segment's scalar-bound phase), tail ops, and output-projection pieces
    of finished lq-blocks.
  - All matmul/E dtypes bf16 (fp8 E or V injects ~3% noise into the signed
    U sums -- it does NOT average out; measured 3.7e-2 rel err).
  - Softmax denominators ride row 64 of U (ones column in V'); 1/r is
    rebuilt across partitions by two K=1 matmuls against ones rows stored
    at partition 64 + a PSUM-direct reciprocal (no cross-partition DMA).
  - Segment-boundary work defers into the next segment's filler slots; the
    single u PSUM accumulator ping-pongs even->odd->even via WAR on its
    DVE eviction.  The last segment's odd-head U runs post-loop split into
    lq halves, each un half evicting behind the other half's matmuls;
    post-loop PSUM evictions alternate ScalarE/VectorE.
  - Output partials leave as bf16 (halves HBM write traffic; host upcasts
    before the 4-core sum, costing ~8e-5 extra rel err).
PSUM budget (8 banks, exact): stE 2 + stO 2 + u 2 + aux 2.
"""

import sys

sys.path.insert(0, "/opt/trn_rl_repo")

import numpy as np

HEADS = 16
D_MODEL = 1024
HD = 64
B = 2
L = 2048
NCORES = 8
HPC = 4          # heads per core
PAIRS = 2        # head pairs per core
NLQB = 4         # lq blocks per core (qat projection granularity)
LQB = L // NLQB  # 512
NLKT = L // 128  # 16 lk tiles
BL = 1024        # lq block per segment

_cache = {}


def _build(has_wbias: bool):
    import concourse.bass as bass  # noqa: F401
    import concourse.tile as tile
    from concourse import bacc, mybir

    f32 = mybir.dt.float32
    f32r = mybir.dt.float32r
    bf16 = mybir.dt.bfloat16
    Exp = mybir.ActivationFunctionType.Exp
    mult = mybir.AluOpType.mult

    nc = bacc.Bacc("TRN2", target_bir_lowering=False, debug=False,
                   num_devices=NCORES)

    qt_d = nc.dram_tensor("qt", [128, PAIRS, L], bf16, kind="ExternalInput")
    kt2e_d = nc.dram_tensor("kt2e", [128, PAIRS, L], bf16,
                            kind="ExternalInput")
    kt2o_d = nc.dram_tensor("kt2o", [128, PAIRS, L], bf16,
                            kind="ExternalInput")
    v_d = nc.dram_tensor("v", [128, HPC, NLKT, 65], bf16,
                         kind="ExternalInput")
    a_d = nc.dram_tensor("a", [128, 2, 128], bf16, kind="ExternalInput")
    wvts_d = nc.dram_tensor("wvts", [64, 2, 128], f32r, kind="ExternalInput")
    ones64_d = nc.dram_tensor("ones64", [65, 2, 128], f32r,
                              kind="ExternalInput")
    wot_d = nc.dram_tensor("wot", [128, PAIRS, D_MODEL], bf16,
                           kind="ExternalInput")
    if has_wbias:
        wb_d = nc.dram_tensor("wb", [128, HPC, NLKT], f32,
                              kind="ExternalInput")
    out_d = nc.dram_tensor("out", [L, D_MODEL], bf16,
                           kind="ExternalOutput")

    NSEG = (L // BL) * PAIRS  # 4 segments: (b, p) = (0,0),(0,1),(1,0),(1,1)

    with tile.TileContext(nc) as tc:
        with (
            tc.tile_pool(name="big", bufs=1) as big,
            tc.tile_pool(name="epool", bufs=40) as epool,
            tc.tile_pool(name="small", bufs=2) as small,
            tc.tile_pool(name="stg", bufs=4) as stgp,
            tc.tile_pool(name="stp", bufs=1, space="PSUM") as stp,
            tc.tile_pool(name="up", bufs=1, space="PSUM") as up,
            tc.tile_pool(name="auxp", bufs=2, space="PSUM") as auxp,
        ):
            # ---- loads, earliest-needed first; split so the first
            # segment's compute starts after ~1MB instead of ~3MB
            a_sb = big.tile([128, 2, 128], bf16)
            nc.sync.dma_start(a_sb[:], a_d[:])
            qt_sb = big.tile([128, PAIRS, L], bf16)
            nc.sync.dma_start(qt_sb[:, 0, 0:LQB], qt_d[:, 0, 0:LQB])
            nc.sync.dma_start(qt_sb[:, 0, LQB:BL], qt_d[:, 0, LQB:BL])
            kt2e_sb = big.tile([128, PAIRS, L], bf16)
            nc.sync.dma_start(kt2e_sb[:, 0, :], kt2e_d[:, 0, :])
            kt2o_sb = big.tile([128, PAIRS, L], bf16)
            nc.sync.dma_start(kt2o_sb[:, 0, :], kt2o_d[:, 0, :])
            v_sb = big.tile([128, HPC, NLKT, 65], bf16)
            nc.sync.dma_start(v_sb[:, 0:1], v_d[:, 0:1])
            nc.sync.dma_start(qt_sb[:, 0, BL:L], qt_d[:, 0, BL:L])
            nc.sync.dma_start(qt_sb[:, 1, :], qt_d[:, 1, :])
            nc.sync.dma_start(v_sb[:, 1:2], v_d[:, 1:2])
            nc.sync.dma_start(kt2e_sb[:, 1, :], kt2e_d[:, 1, :])
            nc.sync.dma_start(kt2o_sb[:, 1, :], kt2o_d[:, 1, :])
            nc.sync.dma_start(v_sb[:, 2:4], v_d[:, 2:4])
            ones64_sb = big.tile([65, 2, 128], f32r)
            nc.sync.dma_start(ones64_sb[:], ones64_d[:])
            wvts_sb = big.tile([64, 2, 128], f32r)
            nc.sync.dma_start(wvts_sb[:], wvts_d[:])
            wot_sb = big.tile([128, PAIRS, D_MODEL], bf16)
            nc.sync.dma_start(wot_sb[:], wot_d[:])
            if has_wbias:
                wb_sb = big.tile([128, HPC, NLKT], f32)
                nc.sync.dma_start(wb_sb[:], wb_d[:])

            # ---- Qa^T = A'-projection of Q^T (block-diagonal A' projects
            # both heads of a pair in one K=128 matmul).  Only the two
            # blocks the first segment needs run upfront; the rest defer
            # into segment 0's filler slots so they never sit ahead of the
            # first S^T matmuls in the PE queue.
            qat2e_sb = big.tile([128, PAIRS, L], bf16)
            qat2o_sb = big.tile([128, PAIRS, L], bf16)
            _qat_dst = [qat2e_sb, qat2o_sb]

            def emit_qat(h, p, j):
                # a_sb[:, h, :] maps q -> [A'q_h ; A'q_h] (duplicated on
                # both partition halves so each lq-half S^T matmul can use
                # its own PE row group and the pair co-issues)
                sl = slice(j * LQB, (j + 1) * LQB)
                qp = auxp.tile([128, LQB], f32, tag="aux",
                               name=f"qp{h}_{p}_{j}")
                nc.tensor.matmul(qp[:], a_sb[:, h, :], qt_sb[:, p, sl],
                                 start=True, stop=True)
                nc.vector.tensor_copy(_qat_dst[h][:, p, sl], qp[:])

            for j in range(2):
                for h in range(2):
                    emit_qat(h, 0, j)
            # deferred: remaining 12 (h, p, j) pieces in seg-0 slots

            otn_sb = [big.tile([128, L], bf16, tag=f"otn{p}",
                               name=f"otn{p}") for p in range(PAIRS)]

            # ---- tail helpers -------------------------------------------
            def emit_tail_half(pv, bv, un, i):
                """1/r broadcast + Wv fold + normalize, one lq half.

                Denominators live in un[h] row 64 (the V' ones column); two
                K=1 matmuls broadcast r_even into PSUM rows 0:64 and r_odd
                into rows 64:128 (accumulating into one bank), then one DVE
                reciprocal evicts 1/r for the normalize STT.  No partition
                DMA, no [2,BL] staging."""
                csl = slice(i * LQB, (i + 1) * LQB)
                rb = auxp.tile([128, LQB], f32, tag="aux",
                               name=f"rb{bv}_{pv}_{i}")
                nc.tensor.matmul(rb[:], ones64_sb[64:65, 0, :],
                                 un[0][64:65, csl],
                                 start=True, stop=False,
                                 tile_position=(64, 0))
                nc.tensor.matmul(rb[:], ones64_sb[64:65, 1, :],
                                 un[1][64:65, csl],
                                 start=False, stop=True,
                                 tile_position=(64, 0))
                rbs = small.tile([128, LQB], f32, tag="rbs",
                                 name=f"rbs{bv}_{pv}_{i}")
                nc.vector.reciprocal_approx_fast(out=rbs[:], in_=rb[:])
                ot = auxp.tile([128, LQB], f32, tag="aux",
                               name=f"ot{bv}_{pv}_{i}")
                nc.tensor.matmul(ot[:], wvts_sb[:, 0, :],
                                 un[0][0:64, csl], start=True, stop=False)
                nc.tensor.matmul(ot[:], wvts_sb[:, 1, :],
                                 un[1][0:64, csl], start=False, stop=True)
                nc.vector.scalar_tensor_tensor(
                    out=otn_sb[pv][:, bv * BL + i * LQB:
                                   bv * BL + (i + 1) * LQB],
                    in0=ot[:], scalar=1.0, in1=rbs[:], op0=mult, op1=mult)

            def emit_proj_piece(bv, lt, scalar_evict=False):
                """Output projection for one 128-row lq tile of block bv."""
                l0 = bv * BL + lt * 128
                for nh in range(2):
                    nsl = slice(nh * 512, (nh + 1) * 512)
                    pp = auxp.tile([128, 512], f32, tag="aux",
                                   name=f"pp{bv}_{lt}_{nh}")
                    nc.tensor.matmul(pp[:], otn_sb[0][:, l0:l0 + 128],
                                     wot_sb[:, 0, nsl],
                                     start=True, stop=False)
                    nc.tensor.matmul(pp[:], otn_sb[1][:, l0:l0 + 128],
                                     wot_sb[:, 1, nsl],
                                     start=False, stop=True)
                    stg = stgp.tile([128, 512], bf16, tag="stg",
                                    name=f"stg{bv}_{lt}_{nh}")
                    if scalar_evict and nh == 0:
                        # post-loop only: ScalarE is idle, let it carry
                        # half the PSUM evictions
                        nc.scalar.copy(stg[:], pp[:])
                    else:
                        nc.vector.tensor_copy(stg[:], pp[:])
                    nc.sync.dma_start(out_d[l0:l0 + 128, nsl], stg[:])

            def emit_u_mms(u, hv, chunks, e_chunks, is_first, is_last):
                """U accumulation matmuls for lk chunks (2 MMs per chunk)."""
                for tc_ in chunks:
                    for i in range(2):
                        isl = slice(i * LQB, (i + 1) * LQB)
                        nc.tensor.matmul(
                            u[:, isl], v_sb[:, hv, tc_, :],
                            e_chunks[tc_][:, isl],
                            start=(is_first and tc_ == 0),
                            stop=(is_last and tc_ == NLKT - 1))

            def finalize_u(u, bv, pv, hh):
                """Evict U rows 0:65 (row 64 = the softmax denominators)."""
                un = small.tile([65, BL], f32r, tag="un",
                                name=f"un{bv}_{pv}_{hh}", bufs=4)
                nc.vector.tensor_copy(un[:], u[0:65, :])
                return un

            # ---- main loop ----------------------------------------------
            # prev: state of segment s-1 {b, p, eE, eO, u_even, drain, un,
            #        u_odd}
            prev = None
            for si in range(NSEG):
                b, p = si // PAIRS, si % PAIRS
                eE = [epool.tile([128, BL], bf16, tag="e",
                                 name=f"eE{b}_{p}_{t}") for t in range(NLKT)]
                eO = [epool.tile([128, BL], bf16, tag="e",
                                 name=f"eO{b}_{p}_{t}") for t in range(NLKT)]
                # even-U chunk schedule: segment 0 has no odd-prev fillers,
                # so it spreads its even U over all slots (denser first
                # half keeps the PE's HAM clock warm); later segments pack
                # it into t=8..14 after the odd-prev U finishes
                if si == 0:
                    even_sched = {t: [t - 1] for t in range(1, NLKT)}
                    drain = [15]
                else:
                    even_sched = {8: [0, 1]}
                    for t_ in range(9, 15):
                        even_sched[t_] = [2 * (t_ - 8), 2 * (t_ - 8) + 1]
                    drain = [14, 15]
                u_even = None
                for t in range(NLKT):
                    # -- S^T row-packed pair matmuls: E,E then O,O
                    stE = stp.tile([128, BL], f32, tag="stE",
                                   name=f"stE{b}_{p}_{t}")
                    stO = stp.tile([128, BL], f32, tag="stO",
                                   name=f"stO{b}_{p}_{t}")
                    ksl = slice(t * 128, (t + 1) * 128)
                    for i in range(2):
                        csl = slice(i * LQB, (i + 1) * LQB)
                        qsl = slice(b * BL + i * LQB, b * BL + (i + 1) * LQB)
                        hsl = slice(64 * i, 64 * i + 64)
                        nc.tensor.matmul(stE[:, csl], kt2e_sb[hsl, p, ksl],
                                         qat2e_sb[hsl, p, qsl],
                                         start=True, stop=True,
                                         tile_position=(64 * i, 0))
                    for i in range(2):
                        csl = slice(i * LQB, (i + 1) * LQB)
                        qsl = slice(b * BL + i * LQB, b * BL + (i + 1) * LQB)
                        hsl = slice(64 * i, 64 * i + 64)
                        nc.tensor.matmul(stO[:, csl], kt2o_sb[hsl, p, ksl],
                                         qat2o_sb[hsl, p, qsl],
                                         start=True, stop=True,
                                         tile_position=(64 * i, 0))
                    # -- exp fused into the PSUM eviction (ScalarE wall)
                    biasE = (wb_sb[:, 2 * p, t:t + 1] if has_wbias else 0.0)
                    biasO = (wb_sb[:, 2 * p + 1, t:t + 1]
                             if has_wbias else 0.0)
                    nc.scalar.activation(eE[t][:], stE[:], Exp, bias=biasE)
                    nc.scalar.activation(eO[t][:], stO[:], Exp, bias=biasO)
                    # -- fillers (strictly after the EXPs: nothing here may
                    # ever sit ahead of the next S^T in the PE queue)
                    if t == 0:
                        if prev is not None:
                            # drain prev even-U tail chunks + evict
                            emit_u_mms(prev["u_even"], 2 * prev["p"],
                                       prev["drain"], prev["eE"],
                                       False, True)
                            prev["un"] = {0: finalize_u(
                                prev["u_even"], prev["b"], prev["p"], 0)}
                    else:
                        if prev is not None and 1 <= t <= 7:
                            if t == 1:
                                prev["u_odd"] = up.tile(
                                    [65, BL], f32, tag="u",
                                    name=f"uO{prev['b']}_{prev['p']}")
                            emit_u_mms(prev["u_odd"], 2 * prev["p"] + 1,
                                       [2 * (t - 1), 2 * t - 1],
                                       prev["eO"], True, False)
                        if prev is not None and t == 8:
                            emit_u_mms(prev["u_odd"], 2 * prev["p"] + 1,
                                       [14, 15], prev["eO"], False, True)
                            prev["un"][1] = finalize_u(
                                prev["u_odd"], prev["b"], prev["p"], 1)
                        if t in even_sched:
                            if u_even is None:
                                u_even = up.tile([65, BL], f32, tag="u",
                                                 name=f"uE{b}_{p}")
                            emit_u_mms(u_even, 2 * p, even_sched[t], eE,
                                       even_sched[t][0] == 0, False)
                        if si == 0 and 1 <= t <= 6:
                            for args in [
                                [(0, 0, 2), (1, 0, 2)],
                                [(0, 0, 3), (1, 0, 3)],
                                [(0, 1, 0), (1, 1, 0)],
                                [(0, 1, 1), (1, 1, 1)],
                                [(0, 1, 2), (1, 1, 2)],
                                [(0, 1, 3), (1, 1, 3)],
                            ][t - 1]:
                                emit_qat(*args)
                        if prev is not None:
                            if t == 11:
                                emit_tail_half(prev["p"], prev["b"],
                                               prev["un"], 0)
                            elif t == 12:
                                emit_tail_half(prev["p"], prev["b"],
                                               prev["un"], 1)
                            elif t >= 13 and p == 0 and b >= 1:
                                # proj of finished block b-1: lt 0..5 here,
                                # lt 6,7 spill into the next segment
                                for lt in range(2 * (t - 13),
                                                2 * (t - 13) + 2):
                                    emit_proj_piece(b - 1, lt)
                            if p == 1 and b >= 1 and t in (5, 6):
                                emit_proj_piece(b - 1, 1 + t)
                        if si == NSEG - 1 and t == 15:
                            # last segment: drain + evict even U now so
                            # the post-loop odd burst starts immediately
                            emit_u_mms(u_even, 2 * p, drain, eE,
                                       False, True)
                            un_last = {0: finalize_u(u_even, b, p, 0)}
                prev = {"b": b, "p": p, "eE": eE, "eO": eO,
                        "u_even": u_even, "drain": drain, "un": None,
                        "u_odd": None}

            # ---- post-loop: last segment's odd head + tails + proj(1).
            # The odd-U burst runs i=0 then i=1; each un half evicts
            # behind the other half's matmuls so the tail chain never
            # stalls the PE queue.
            prev["un"] = un_last
            u_odd = up.tile([65, BL], f32, tag="u", name="uO_last")
            unO = None
            for i in range(2):
                isl = slice(i * LQB, (i + 1) * LQB)
                for tc_ in range(NLKT):
                    nc.tensor.matmul(u_odd[:, isl],
                                     v_sb[:, 2 * prev["p"] + 1, tc_, :],
                                     prev["eO"][tc_][:, isl],
                                     start=(tc_ == 0), stop=(tc_ == 15))
                if unO is None:
                    unO = small.tile([65, BL], f32r, tag="un",
                                     name="unO_last", bufs=4)
                nc.vector.tensor_copy(unO[:, isl], u_odd[0:65, isl])
            prev["un"][1] = unO
            emit_tail_half(prev["p"], prev["b"], prev["un"], 0)
            for lt in range(0, 4):
                emit_proj_piece(1, lt, scalar_evict=True)
            emit_tail_half(prev["p"], prev["b"], prev["un"], 1)
            for lt in range(4, 8):
                emit_proj_piece(1, lt, scalar_evict=True)
    nc.compile()
    return nc


def _get_nc(has_wbias: bool):
    key = ("nc", has_wbias)
    if key not in _cache:
        _cache[key] = _build(has_wbias)
    return _cache[key]


def _prep_inputs(values, keys, query, Wq, bq, Wk, bk, Wv, bv, Wo, bo):
    """Host-side shard/layout prep. Returns (in_maps, bo_eff, has_wbias)."""
    f32 = np.float32
    values = np.asarray(values, f32)
    keys = np.asarray(keys, f32)
    query = np.asarray(query, f32)
    Wq = np.asarray(Wq, f32)
    bq = np.asarray(bq, f32)
    Wk = np.asarray(Wk, f32)
    bk = np.asarray(bk, f32)  # noqa: F841  (cancels in softmax)
    Wv = np.asarray(Wv, f32)
    bv = np.asarray(bv, f32)
    Wo = np.asarray(Wo, f32)
    bo = np.asarray(bo, f32)

    import ml_dtypes
    bf = ml_dtypes.bfloat16
    a0 = (Wq.T @ Wk / 32.0).astype(f32)         # [d, e]
    # a[:, 0, :]: q -> [A'q_E ; A'q_E]   a[:, 1, :]: q -> [A'q_O ; A'q_O]
    # (lhsT convention: out[m] = sum_k a[k, m] * q[k])
    a = np.zeros((128, 2, 128), bf)
    a[0:64, 0, 0:64] = a0
    a[0:64, 0, 64:128] = a0
    a[64:128, 1, 0:64] = a0
    a[64:128, 1, 64:128] = a0
    wvts = np.zeros((64, 2, 128), f32)
    wvts[:, 0, 0:64] = Wv.T
    wvts[:, 1, 64:128] = Wv.T
    ones64 = np.zeros((65, 2, 128), f32)
    ones64[64, 0, 0:64] = 1.0
    ones64[64, 1, 64:128] = 1.0
    # bv contributes a constant row: fold into bo
    bo_eff = bo + Wo @ np.tile(bv, HEADS)

    has_wbias = bool(np.any(bq != 0.0))
    if has_wbias:
        m = (Wk.T @ bq / 32.0).astype(f32)      # [d]
        kh = keys.reshape(B, L, HEADS, HD)
        w_all = np.einsum("blhd,d->bhl", kh, m).astype(f32)

    qh = query.reshape(B, L, HEADS, HD)
    khds = keys.reshape(B, L, HEADS, HD)
    vh = values.reshape(B, L, HEADS, HD)

    in_maps = []
    for c in range(NCORES):
        b = c // 4
        h0 = 4 * (c % 4)
        hs = list(range(h0, h0 + HPC))
        # [128, PAIRS, L]: head pair stacked on partitions (mirrors kt)
        qt = np.empty((128, PAIRS, L), bf)
        for p in range(PAIRS):
            qt[0:64, p, :] = qh[b, :, hs[2 * p], :].T
            qt[64:128, p, :] = qh[b, :, hs[2 * p + 1], :].T
        # per-head K^T duplicated on both partition halves (row-group
        # co-issue of the two lq-half S^T matmuls)
        kt2e = np.empty((128, PAIRS, L), bf)
        kt2o = np.empty((128, PAIRS, L), bf)
        for p in range(PAIRS):
            ke = khds[b, :, hs[2 * p], :].T
            ko = khds[b, :, hs[2 * p + 1], :].T
            kt2e[0:64, p, :] = ke
            kt2e[64:128, p, :] = ke
            kt2o[0:64, p, :] = ko
            kt2o[64:128, p, :] = ko
        v = np.empty((128, HPC, NLKT, 65), bf)
        for i in range(HPC):
            v[:, i, :, 0:64] = vh[b, :, hs[i], :].reshape(
                NLKT, 128, HD).transpose(1, 0, 2)
        v[:, :, :, 64] = 1.0
        wot = np.empty((128, PAIRS, D_MODEL), bf)
        for p in range(PAIRS):
            wot[0:64, p, :] = Wo[:, hs[2 * p] * HD:(hs[2 * p] + 1) * HD].T
            wot[64:128, p, :] = Wo[:, hs[2 * p + 1] * HD:
                                   (hs[2 * p + 1] + 1) * HD].T
        im = {
            "qt": qt,
            "kt2e": kt2e,
            "kt2o": kt2o,
            "v": v,
            "a": a,
            "wvts": wvts,
            "ones64": ones64,
            "wot": wot,
        }
        if has_wbias:
            wb = np.empty((128, HPC, NLKT), f32)
            for i in range(HPC):
                wb[:, i, :] = w_all[b, hs[i]].reshape(NLKT, 128).T
            im["wb"] = wb
        in_maps.append(im)
    return in_maps, bo_eff, has_wbias


def kernel(values, keys, query, Wq, bq, Wk, bk, Wv, bv, Wo, bo,
           _trace=False):
    from concourse.bass_utils import run_bass_kernel_spmd

    in_maps, bo_eff, has_wbias = _prep_inputs(
        values, keys, query, Wq, bq, Wk, bk, Wv, bv, Wo, bo)
    nc = _get_nc(has_wbias)
    kwargs = {}
    if _trace:
        kwargs = dict(trace=True, trace_cores=[0])
    res = run_bass_kernel_spmd(nc, in_maps, core_ids=list(range(NCORES)),
                               **kwargs)
    out = np.empty((B, L, D_MODEL), np.float32)
    for b in range(B):
        acc = res.results[4 * b]["out"].astype(np.float64)
        for i in range(1, 4):
            acc += res.results[4 * b + i]["out"]
        out[b] = (acc + bo_eff).astype(np.float32)
    if _trace:
        kernel.last_exec_time_ns = res.exec_time_ns
        kernel.last_trace = res.instructions_and_trace
    return out
